# revision 10
# baseline (speedup 1.0000x reference)
"""Trainium2 Bass kernel: row-wise cosine similarity discriminator.

Computes, for full inputs s, h_rl, h_fk of shape [B=8, N=8192, D=512] f32:
    out = concat(rowdot(l2n(s), l2n(h_rl)), rowdot(l2n(s), l2n(h_fk)), axis=1)
with l2n(x) = x / max(||x||_2, 1e-12), giving out shape [8, 16384] f32.

Sharding: pure data parallel over batch B — core b processes batch b.

Per-core kernel strategy (memory-bound: 48 MiB input / core; HW-measured
DMA stream rate 380 GB/s -> ~132 us DMA floor):
  - row mapping row = p*NT + t: partition p holds NT=64 consecutive DRAM
    rows, so chunked loads [128, J, 512] are J*2 KiB CONTIGUOUS per
    partition (large DMA descriptors), and the final stats tile [P, NT]
    stores to DRAM directly with no transpose (out[k] viewed as [P, NT]).
  - loads are SWDGE (gpsimd) dma_start with inline f32->fp16 cast —
    HW-measured at the same 380 GB/s as HWDGE f32, and the fp16 tiles
    unlock the DVE 2x_1P mode for tensor_tensor (16-bit, step 1).
  - engine split (HW-measured per-row-tile costs: ACT fused Square+
    accum_out 906ns, ACT batched square 463ns, GpSimd tensor_tensor
    1100ns = its ~2.15 ns/elem architectural floor, DVE tensor_tensor
    fp16 287ns, DVE batched reduce 541ns; DVE tensor_reduce only has a
    1x uop), min-max balanced just under the DMA floor:
      ACT    s^2 and h_rl^2 fused Square+accum_out; h_fk^2 tiles j>=6J/8
      GpSimd DMA emission; p_rl = s*h_rl; h_fk^2 tiles j<3J/8
      DVE    p_fk = s*h_fk (2x); h_fk^2 tiles 3J/8<=j<6J/8; all 3
             batched reductions (p_rl, hh_fk, p_fk)
  - variable chunk sizes (small first chunks for fast pipeline ramp,
    small last chunks for a short tail after the final DMA)
  - finals (sqrt on ACT; clamp/reciprocal/scale on DVE — gpsimd costs
    ~1us per tiny op) on [128, 64] stats tiles
  - this walrus build cannot encode multi-wait Drain/STT instructions:
    _fix_tail_drain_waits() rewrites multi-wait instructions into
    single-wait EventSemaphores
"""

import numpy as np

import concourse.bass as bass
import concourse.mybir as mybir
import concourse.tile as tile
from concourse.bass_utils import run_bass_kernel_spmd

B, N, D = 8, 8192, 512
P = 128                    # SBUF partitions
NT = N // P                # 64 rows per partition (row = p*NT + t)
JMAX = 8                   # max row-tiles per chunk ([P, J, D] per DMA)
# chunk sizes: fast ramp, big middle, short tail; sums to NT
CHUNKS = [2, 2, 4] + [8] * 6 + [4, 2, 1, 1]
assert sum(CHUNKS) == NT
EPS = 1e-12
F32 = mybir.dt.float32
FP16 = mybir.dt.float16


def _fix_tail_drain_waits(nc):
    """This image's walrus cannot encode more than one sem wait on several
    instruction kinds (Tile's end-of-kernel Drain, STT, ...). Move each
    wait of any multi-wait instruction onto its own EventSemaphore
    inserted right before it on the same engine — identical semantics
    (engine program order), always encodable."""
    for fn in nc.m.functions:
        for bb in fn.blocks:
            new = []
            for inst in bb.instructions:
                si = inst.sync_info
                if (
                    not isinstance(inst, mybir.InstEventSemaphore)
                    and si is not None
                    and si.on_wait
                    and len(si.on_wait) > 1
                ):
                    for k, w in enumerate(list(si.on_wait)):
                        ev = mybir.InstEventSemaphore(
                            name=f"{inst.name}-prewait{k}", ins=[], outs=[]
                        )
                        ev.engine = inst.engine
                        ev.sync_info = mybir.SyncInfo(on_wait=[w], on_update=[])
                        new.append(ev)
                    inst.sync_info = mybir.SyncInfo(
                        on_wait=[], on_update=list(si.on_update)
                    )
                new.append(inst)
            bb.instructions[:] = new


def build_nc():
    nc = bass.Bass(trn_type="TRN2")
    s_h = nc.declare_dram_parameter("s", [N, D], F32, isOutput=False)
    hrl_h = nc.declare_dram_parameter("h_rl", [N, D], F32, isOutput=False)
    hfk_h = nc.declare_dram_parameter("h_fk", [N, D], F32, isOutput=False)
    # out[k][p, t] = score of row p*NT + t  ->  flat [2, N] row-major
    out_h = nc.declare_dram_parameter("out", [2, P, NT], F32, isOutput=True)

    # DRAM view: row p*NT + t  ->  [p, t, d]; per-partition rows contiguous
    def rows(h):
        return h[:, :].rearrange("(p t) d -> p t d", p=P, t=NT)

    s_g, h1_g, h2_g = rows(s_h), rows(hrl_h), rows(hfk_h)

    Sq = mybir.ActivationFunctionType.Square
    Red = dict(axis=mybir.AxisListType.X, op=mybir.AluOpType.add)
    Mult = mybir.AluOpType.mult

    with tile.TileContext(nc) as tc:
        with (
            tc.tile_pool(name="ins", bufs=2) as ins,
            tc.tile_pool(name="scr", bufs=2) as scr,
            tc.tile_pool(name="stats", bufs=1) as stats,
            tc.tile_pool(name="fin", bufs=1) as fin,
        ):
            # per-row accumulators, column t = row's slot in its partition
            # stats_q: [ss, hh_rl, hh_fk]; stats_p: [sp_rl, sp_fk]
            stats_q = stats.tile([P, 3, NT], F32, tag="stats_q")
            stats_p = stats.tile([P, 2, NT], F32, tag="stats_p")
            ss, hh_rl, hh_fk = (stats_q[:, k, :] for k in range(3))
            sp_rl, sp_fk = (stats_p[:, k, :] for k in range(2))

            # junk sink for the fused squares' elementwise outputs (the
            # accum_out is what we keep); WAW on it is same-engine serial
            junk = fin.tile([P, D], FP16, tag="junk")

            t0 = 0
            for J in CHUNKS:
                cols = slice(t0, t0 + J)
                s_f = ins.tile([P, JMAX, D], FP16, tag="s")
                h1_f = ins.tile([P, JMAX, D], FP16, tag="h1")
                h2_f = ins.tile([P, JMAX, D], FP16, tag="h2")
                q2_f = scr.tile([P, JMAX, D], FP16, tag="q2")
                p1_f = scr.tile([P, JMAX, D], FP16, tag="p1")
                p2_f = scr.tile([P, JMAX, D], FP16, tag="p2")
                s_t, h1_t, h2_t = s_f[:, :J, :], h1_f[:, :J, :], h2_f[:, :J, :]
                q2, p1, p2 = q2_f[:, :J, :], p1_f[:, :J, :], p2_f[:, :J, :]

                # SWDGE loads with inline f32->fp16 cast (emission is a
                # cheap gpsimd queue op; the stream itself is line-rate)
                nc.gpsimd.dma_start(out=s_t, in_=s_g[:, cols, :])
                nc.gpsimd.dma_start(out=h1_t, in_=h1_g[:, cols, :])
                nc.gpsimd.dma_start(out=h2_t, in_=h2_g[:, cols, :])

                # ACT: fused square+reduce per row-tile (s^2 then h_rl^2,
                # in DMA arrival order)
                for j in range(J):
                    nc.scalar.activation(
                        out=junk, in_=s_t[:, j, :], func=Sq,
                        accum_out=stats_q[:, 0, t0 + j:t0 + j + 1])
                for j in range(J):
                    nc.scalar.activation(
                        out=junk, in_=h1_t[:, j, :], func=Sq,
                        accum_out=stats_q[:, 1, t0 + j:t0 + j + 1])

                # h_fk^2 split three ways: gpsimd / DVE / ACT
                js1 = (3 * J) // 8
                js2 = (6 * J) // 8
                # GpSimd: p_rl product, then its share of h_fk^2
                nc.gpsimd.tensor_tensor(out=p1, in0=s_t, in1=h1_t, op=Mult)
                if js1 > 0:
                    nc.gpsimd.tensor_tensor(
                        out=q2[:, :js1, :], in0=h2_t[:, :js1, :],
                        in1=h2_t[:, :js1, :], op=Mult)
                if js2 < J:
                    nc.scalar.activation(
                        out=q2[:, js2:, :], in_=h2_t[:, js2:, :], func=Sq)

                # DVE: p_fk product (fp16 2x), its h_fk^2 share, reductions
                nc.vector.tensor_tensor(out=p2, in0=s_t, in1=h2_t, op=Mult)
                if js1 < js2:
                    nc.vector.tensor_tensor(
                        out=q2[:, js1:js2, :], in0=h2_t[:, js1:js2, :],
                        in1=h2_t[:, js1:js2, :], op=Mult)
                nc.vector.tensor_reduce(out=stats_p[:, 0, cols], in_=p1, **Red)
                nc.vector.tensor_reduce(out=stats_p[:, 1, cols], in_=p2, **Red)
                nc.vector.tensor_reduce(out=stats_q[:, 2, cols], in_=q2, **Red)
                t0 += J

            # ---- finals on [P, NT] stats tiles (sqrt on ACT; everything
            # else on DVE — reciprocal must be DVE anyway, and gpsimd
            # costs ~1us per tiny op) ----
            Sqrt = mybir.ActivationFunctionType.Sqrt
            ns = fin.tile([P, NT], F32, tag="ns")
            n1 = fin.tile([P, NT], F32, tag="n1")
            n2 = fin.tile([P, NT], F32, tag="n2")
            nc.scalar.activation(out=ns, in_=ss, func=Sqrt)
            nc.scalar.activation(out=n1, in_=hh_rl, func=Sqrt)
            nc.scalar.activation(out=n2, in_=hh_fk, func=Sqrt)
            nc.vector.tensor_scalar_max(ns, ns, EPS)
            nc.vector.tensor_scalar_max(n1, n1, EPS)
            nc.vector.tensor_scalar_max(n2, n2, EPS)
            den1 = fin.tile([P, NT], F32, tag="den1")
            den2 = fin.tile([P, NT], F32, tag="den2")
            nc.vector.tensor_tensor(den1, ns, n1, op=Mult)
            nc.vector.tensor_tensor(den2, ns, n2, op=Mult)
            nc.vector.reciprocal(den1, den1)
            nc.vector.reciprocal(den2, den2)
            o1 = fin.tile([P, NT], F32, tag="o1")
            o2 = fin.tile([P, NT], F32, tag="o2")
            nc.vector.tensor_tensor(o1, sp_rl, den1, op=Mult)
            nc.vector.tensor_tensor(o2, sp_fk, den2, op=Mult)
            nc.sync.dma_start(out=out_h[0], in_=o1)
            nc.sync.dma_start(out=out_h[1], in_=o2)

    _fix_tail_drain_waits(nc)
    return nc


_NC_CACHE = None


def kernel(s, h_rl, h_fk, trace=False):
    global _NC_CACHE
    s = np.ascontiguousarray(np.asarray(s, dtype=np.float32))
    h_rl = np.ascontiguousarray(np.asarray(h_rl, dtype=np.float32))
    h_fk = np.ascontiguousarray(np.asarray(h_fk, dtype=np.float32))
    assert s.shape == (B, N, D), s.shape

    if _NC_CACHE is None:
        _NC_CACHE = build_nc()
    nc = _NC_CACHE

    in_maps = [
        {"s": s[b], "h_rl": h_rl[b], "h_fk": h_fk[b]} for b in range(B)
    ]
    res = run_bass_kernel_spmd(nc, in_maps, core_ids=list(range(B)), trace=trace)
    out = np.empty((B, 2 * N), dtype=np.float32)
    for b in range(B):
        o = res.results[b]["out"]  # [2, P, NT]; row p*NT+t -> o[k].ravel()
        out[b, :N] = o[0].reshape(N)
        out[b, N:] = o[1].reshape(N)
    if trace:
        return out, res
    return out


# revision 12
# speedup vs baseline: 1.0751x; 1.0751x over previous
"""Trainium2 Bass kernel: row-wise cosine similarity discriminator.

Computes, for full inputs s, h_rl, h_fk of shape [B=8, N=8192, D=512] f32:
    out = concat(rowdot(l2n(s), l2n(h_rl)), rowdot(l2n(s), l2n(h_fk)), axis=1)
with l2n(x) = x / max(||x||_2, 1e-12), giving out shape [8, 16384] f32.

Sharding: pure data parallel over batch B — core b processes batch b.

Per-core kernel strategy (memory-bound: 48 MiB input / core; HW-measured
DMA stream rate 380 GB/s -> ~132 us DMA floor):
  - row mapping row = p*NT + t: partition p holds NT=64 consecutive DRAM
    rows, so chunked loads [128, J, 512] are contiguous per partition,
    and the final stats tile [P, NT] stores to DRAM with no transpose.
  - loads are SWDGE (gpsimd) dma_start with inline f32->fp16 cast —
    HW-measured at the same 380 GB/s as HWDGE f32. Emissions are
    software-pipelined TWO chunks ahead of compute on the gpsimd queue
    (ins pool bufs=3) so the DMA ring never starves behind gpsimd
    compute (program order would otherwise serialize stream and TT).
  - engine split (HW-measured: ACT fused Square+accum_out 906ns/row-tile
    incl. accumulator read; GpSimd tensor_tensor ~2.15 ns/elem; DVE
    fp16 tensor_tensor ~0.84 ns/elem; DVE tensor_reduce 1x-only, so
    reductions use two fp16 2x fold passes + a 1x reduce of the
    128-wide remainder):
      ACT    s^2 fused; h_rl^2 half fused, half batched; h_fk^2 tiles
             j>=J/2 batched
      GpSimd DMA emissions; p_rl = s*h_rl; h_fk^2 tiles j<J/2
      DVE    p_fk = s*h_fk; fold+reduce for p_rl, p_fk, hh_fk and the
             batched h_rl^2 half
  - finals (sqrt on ACT; clamp/reciprocal/scale on DVE) on [128, 64]
    stats tiles
  - this walrus build cannot encode multi-wait Drain/STT instructions:
    _fix_tail_drain_waits() rewrites multi-wait instructions into
    single-wait EventSemaphores
"""

import numpy as np

import concourse.bass as bass
import concourse.mybir as mybir
import concourse.tile as tile
from concourse.bass_utils import run_bass_kernel_spmd

B, N, D = 8, 8192, 512
P = 128                    # SBUF partitions
NT = N // P                # 64 rows per partition (row = p*NT + t)
JMAX = 8                   # max row-tiles per chunk ([P, J, D] per DMA)
# chunk sizes: fast ramp, big middle, short tail; sums to NT
CHUNKS = [2, 2, 4] + [8] * 6 + [4, 2, 1, 1]
assert sum(CHUNKS) == NT
EPS = 1e-12
F32 = mybir.dt.float32
FP16 = mybir.dt.float16


def _fix_tail_drain_waits(nc):
    """This image's walrus cannot encode more than one sem wait on several
    instruction kinds (Tile's end-of-kernel Drain, STT, ...). Move each
    wait of any multi-wait instruction onto its own EventSemaphore
    inserted right before it on the same engine — identical semantics
    (engine program order), always encodable."""
    for fn in nc.m.functions:
        for bb in fn.blocks:
            new = []
            for inst in bb.instructions:
                si = inst.sync_info
                if (
                    not isinstance(inst, mybir.InstEventSemaphore)
                    and si is not None
                    and si.on_wait
                    and len(si.on_wait) > 1
                ):
                    for k, w in enumerate(list(si.on_wait)):
                        ev = mybir.InstEventSemaphore(
                            name=f"{inst.name}-prewait{k}", ins=[], outs=[]
                        )
                        ev.engine = inst.engine
                        ev.sync_info = mybir.SyncInfo(on_wait=[w], on_update=[])
                        new.append(ev)
                    inst.sync_info = mybir.SyncInfo(
                        on_wait=[], on_update=list(si.on_update)
                    )
                new.append(inst)
            bb.instructions[:] = new


def build_nc():
    nc = bass.Bass(trn_type="TRN2")
    s_h = nc.declare_dram_parameter("s", [N, D], F32, isOutput=False)
    hrl_h = nc.declare_dram_parameter("h_rl", [N, D], F32, isOutput=False)
    hfk_h = nc.declare_dram_parameter("h_fk", [N, D], F32, isOutput=False)
    # out[k][p, t] = score of row p*NT + t  ->  flat [2, N] row-major
    out_h = nc.declare_dram_parameter("out", [2, P, NT], F32, isOutput=True)

    # DRAM view: row p*NT + t  ->  [p, t, d]; per-partition rows contiguous
    def rows(h):
        return h[:, :].rearrange("(p t) d -> p t d", p=P, t=NT)

    views = (rows(s_h), rows(hrl_h), rows(hfk_h))

    Sq = mybir.ActivationFunctionType.Square
    Add = mybir.AluOpType.add
    Red = dict(axis=mybir.AxisListType.X, op=Add)
    Mult = mybir.AluOpType.mult
    NC = len(CHUNKS)
    OFFS = [sum(CHUNKS[:c]) for c in range(NC)]

    with tile.TileContext(nc) as tc:
        with (
            tc.tile_pool(name="ins", bufs=3) as ins,
            tc.tile_pool(name="scr", bufs=2) as scr,
            tc.tile_pool(name="fld", bufs=2) as fld,
            tc.tile_pool(name="stats", bufs=1) as stats,
            tc.tile_pool(name="fin", bufs=1) as fin,
        ):
            # per-row accumulators, column t = row's slot in its partition
            # stats_q: [ss, hh_rl, hh_fk]; stats_p: [sp_rl, sp_fk]
            stats_q = stats.tile([P, 3, NT], F32, tag="stats_q")
            stats_p = stats.tile([P, 2, NT], F32, tag="stats_p")
            ss, hh_rl, hh_fk = (stats_q[:, k, :] for k in range(3))
            sp_rl, sp_fk = (stats_p[:, k, :] for k in range(2))

            # junk sink for the fused squares' elementwise outputs (the
            # accum_out is what we keep); WAW on it is same-engine serial
            junk = fin.tile([P, D], FP16, tag="junk")

            tiles = {}

            def emit(c):
                J = CHUNKS[c]
                cols = slice(OFFS[c], OFFS[c] + J)
                tl = []
                for k, tag in enumerate(("s", "h1", "h2")):
                    f = ins.tile([P, JMAX, D], FP16, tag=tag, name=f"in{c}{k}")
                    nc.gpsimd.dma_start(out=f[:, :J, :], in_=views[k][:, cols, :])
                    tl.append(f)
                tiles[c] = tl

            def fold_red(src, out_col, f1, f2, J):
                # src [P, J, 512] fp16 -> fold to [P, J, 128] (2x mode),
                # then 1x tensor_reduce the remainder
                nc.vector.tensor_tensor(
                    out=f1[:, :J, :], in0=src[:, :J, 0:256],
                    in1=src[:, :J, 256:512], op=Add)
                nc.vector.tensor_tensor(
                    out=f2[:, :J, :], in0=f1[:, :J, 0:128],
                    in1=f1[:, :J, 128:256], op=Add)
                nc.vector.tensor_reduce(out=out_col, in_=f2[:, :J, :], **Red)

            # --- software-pipelined main loop: emissions 2 chunks ahead ---
            emit(0)
            emit(1)
            for c in range(NC):
                if c + 2 < NC:
                    emit(c + 2)
                J = CHUNKS[c]
                t0 = OFFS[c]
                cols = slice(t0, t0 + J)
                s_f, h1_f, h2_f = tiles.pop(c)
                s_t, h1_t, h2_t = s_f[:, :J, :], h1_f[:, :J, :], h2_f[:, :J, :]
                q1s_f = scr.tile([P, JMAX // 2, D], FP16, tag="q1s",
                                 name=f"q1s{c}")
                q2_f = scr.tile([P, JMAX, D], FP16, tag="q2", name=f"q2{c}")
                p1_f = scr.tile([P, JMAX, D], FP16, tag="p1", name=f"p1{c}")
                p2_f = scr.tile([P, JMAX, D], FP16, tag="p2", name=f"p2{c}")
                q2, p1, p2 = q2_f[:, :J, :], p1_f[:, :J, :], p2_f[:, :J, :]
                jh = J // 2           # fused/batched split for h_rl^2
                js = J // 2           # gpsimd/ACT split for h_fk^2

                # ACT: s^2 fully fused; h_rl^2 fused for j<jh, batched rest;
                # h_fk^2 batched for j>=js
                for j in range(J):
                    nc.scalar.activation(
                        out=junk, in_=s_t[:, j, :], func=Sq,
                        accum_out=stats_q[:, 0, t0 + j:t0 + j + 1])
                for j in range(jh):
                    nc.scalar.activation(
                        out=junk, in_=h1_t[:, j, :], func=Sq,
                        accum_out=stats_q[:, 1, t0 + j:t0 + j + 1])
                if jh < J:
                    nc.scalar.activation(
                        out=q1s_f[:, :J - jh, :], in_=h1_t[:, jh:, :], func=Sq)
                if js < J:
                    nc.scalar.activation(
                        out=q2[:, js:, :], in_=h2_t[:, js:, :], func=Sq)

                # GpSimd: p_rl product; h_fk^2 tiles j<js
                nc.gpsimd.tensor_tensor(out=p1, in0=s_t, in1=h1_t, op=Mult)
                if js > 0:
                    nc.gpsimd.tensor_tensor(
                        out=q2[:, :js, :], in0=h2_t[:, :js, :],
                        in1=h2_t[:, :js, :], op=Mult)

                # DVE: p_fk product (fp16), then fold+reduce everything
                nc.vector.tensor_tensor(out=p2, in0=s_t, in1=h2_t, op=Mult)
                f1 = fld.tile([P, JMAX, 256], FP16, tag="f1", name=f"f1{c}")
                f2 = fld.tile([P, JMAX, 128], FP16, tag="f2", name=f"f2{c}")
                fold_red(p1, stats_p[:, 0, cols], f1, f2, J)
                f1b = fld.tile([P, JMAX, 256], FP16, tag="f1b", name=f"f1b{c}")
                f2b = fld.tile([P, JMAX, 128], FP16, tag="f2b", name=f"f2b{c}")
                fold_red(p2, stats_p[:, 1, cols], f1b, f2b, J)
                f1c = fld.tile([P, JMAX, 256], FP16, tag="f1c", name=f"f1c{c}")
                f2c = fld.tile([P, JMAX, 128], FP16, tag="f2c", name=f"f2c{c}")
                fold_red(q2, stats_q[:, 2, cols], f1c, f2c, J)
                if jh < J:
                    f1d = fld.tile([P, JMAX // 2, 256], FP16, tag="f1d",
                                   name=f"f1d{c}")
                    f2d = fld.tile([P, JMAX // 2, 128], FP16, tag="f2d",
                                   name=f"f2d{c}")
                    fold_red(q1s_f, stats_q[:, 1, t0 + jh:t0 + J],
                             f1d, f2d, J - jh)

            # ---- finals on [P, NT] stats tiles (sqrt on ACT; everything
            # else on DVE — reciprocal must be DVE anyway, and gpsimd
            # costs ~1us per tiny op) ----
            Sqrt = mybir.ActivationFunctionType.Sqrt
            ns = fin.tile([P, NT], F32, tag="ns")
            n1 = fin.tile([P, NT], F32, tag="n1")
            n2 = fin.tile([P, NT], F32, tag="n2")
            nc.scalar.activation(out=ns, in_=ss, func=Sqrt)
            nc.scalar.activation(out=n1, in_=hh_rl, func=Sqrt)
            nc.scalar.activation(out=n2, in_=hh_fk, func=Sqrt)
            nc.vector.tensor_scalar_max(ns, ns, EPS)
            nc.vector.tensor_scalar_max(n1, n1, EPS)
            nc.vector.tensor_scalar_max(n2, n2, EPS)
            den1 = fin.tile([P, NT], F32, tag="den1")
            den2 = fin.tile([P, NT], F32, tag="den2")
            nc.vector.tensor_tensor(den1, ns, n1, op=Mult)
            nc.vector.tensor_tensor(den2, ns, n2, op=Mult)
            nc.vector.reciprocal(den1, den1)
            nc.vector.reciprocal(den2, den2)
            o1 = fin.tile([P, NT], F32, tag="o1")
            o2 = fin.tile([P, NT], F32, tag="o2")
            nc.vector.tensor_tensor(o1, sp_rl, den1, op=Mult)
            nc.vector.tensor_tensor(o2, sp_fk, den2, op=Mult)
            nc.sync.dma_start(out=out_h[0], in_=o1)
            nc.sync.dma_start(out=out_h[1], in_=o2)

    _fix_tail_drain_waits(nc)
    return nc


_NC_CACHE = None


def kernel(s, h_rl, h_fk, trace=False):
    global _NC_CACHE
    s = np.ascontiguousarray(np.asarray(s, dtype=np.float32))
    h_rl = np.ascontiguousarray(np.asarray(h_rl, dtype=np.float32))
    h_fk = np.ascontiguousarray(np.asarray(h_fk, dtype=np.float32))
    assert s.shape == (B, N, D), s.shape

    if _NC_CACHE is None:
        _NC_CACHE = build_nc()
    nc = _NC_CACHE

    in_maps = [
        {"s": s[b], "h_rl": h_rl[b], "h_fk": h_fk[b]} for b in range(B)
    ]
    res = run_bass_kernel_spmd(nc, in_maps, core_ids=list(range(B)), trace=trace)
    out = np.empty((B, 2 * N), dtype=np.float32)
    for b in range(B):
        o = res.results[b]["out"]  # [2, P, NT]; row p*NT+t -> o[k].ravel()
        out[b, :N] = o[0].reshape(N)
        out[b, N:] = o[1].reshape(N)
    if trace:
        return out, res
    return out
